# revision 1
# baseline (speedup 1.0000x reference)
"""GridRNN kernel for Trainium2 (Bass/Tile), 8-core data-parallel over batch.

Structural insight: in this GridRNN, depth-0 inputs are broadcast (x over j,
y over i) and the carry-roll along j is identity on j-constant carries, so by
induction every grid cell depends on only ONE coordinate:
    out[b,d,i,j,0,:] = f_d(b,i)   (hx, independent of j)
    out[b,d,i,j,1,:] = g_d(b,j)   (hy, independent of i)
with tiny 96-step RNN chains:
    f0(i) = tanh(Wx_ih0^T x_i   + Wx_hh0^T f0(i-1) + bx0),  f0(-1)=0
    f1(i) = tanh(Wx_ih1^T f0(i) + Wx_hh1^T f1(i-1) + bx1)
    g0(j) = tanh(Wy_ih0^T y_j   + Wy_hh0^T g0(j-1) + by0)
    g1(j) = tanh(Wy_ih1^T g0((j-1)%96) + Wy_hh1^T g1(j-1) + by1)
Each core computes one sample's chains (PE matvecs + ACT tanh) and assembles
its 18.9MB output slice with broadcast DMAs (stride-0 source access patterns).
"""

import numpy as np

import concourse.bass as bass
import concourse.bacc as bacc
import concourse.mybir as mybir
import concourse.tile as tile
import concourse.bass_utils as bass_utils

H, S, T, D, B = 128, 96, 96, 2, 8
F32 = mybir.dt.float32
CHUNK = 32
TANH = mybir.ActivationFunctionType.Tanh

WNAMES = ["wx_ih0", "wx_hh0", "wx_ih1", "wx_hh1",
          "wy_ih0", "wy_hh0", "wy_ih1", "wy_hh1"]
BNAMES = ["btx0", "btx1", "bty0", "bty1"]

_off = 0
COLS = {}
for _nm, _w in ([("xT", S), ("yT", T), ("ident", H)]
                + [(n, H) for n in WNAMES] + [(n, 1) for n in BNAMES]):
    COLS[_nm] = (_off, _off + _w)
    _off += _w
NCOLS = _off

_PROG = None


def _build_program(emit_dma=True, emit_tp=True):
    nc = bacc.Bacc("TRN2", target_bir_lowering=False, debug=False)

    c_h = nc.dram_tensor("consts", [H, NCOLS], F32, kind="ExternalInput")
    out_h = nc.dram_tensor("out", [D, S, T, 2, H], F32, kind="ExternalOutput")

    with tile.TileContext(nc) as tc:
        with (
            tc.tile_pool(name="const", bufs=1) as cpool,
            tc.tile_pool(name="chains", bufs=1) as chpool,
            tc.tile_pool(name="nat", bufs=1) as natpool,
            tc.tile_pool(name="bias", bufs=1) as biaspool,
            tc.tile_pool(name="ps", bufs=4, space="PSUM") as pspool,
            tc.tile_pool(name="pst", bufs=2, space="PSUM") as pstpool,
        ):
            cons = cpool.tile([H, NCOLS], F32, tag="consts", name="consts")
            nc.sync.dma_start(cons[:, :], c_h[:, :])

            def sb(nm, j0=0, w=None):
                a, b_ = COLS[nm]
                if w is None:
                    w = b_ - a - j0
                return cons[:, a + j0:a + j0 + w]

            fT = [chpool.tile([H, S], F32, tag=f"f{d}T", name=f"f{d}T") for d in range(D)]
            gT = [chpool.tile([H, T], F32, tag=f"g{d}T", name=f"g{d}T") for d in range(D)]
            fN = [natpool.tile([S, H], F32, tag=f"f{d}N", name=f"f{d}N") for d in range(D)]
            gN = [natpool.tile([T, H], F32, tag=f"g{d}N", name=f"g{d}N") for d in range(D)]

            def tick(dstT, col, w_in, rhs_in, w_hh, prev, bias):
                # input projection via matmul, combined bias via act bias AP
                ps = pspool.tile([H, 1], F32, tag="ps", name="ps")
                only = prev is None
                nc.tensor.matmul(ps[:, 0:1], sb(w_in), rhs_in,
                                 start=True, stop=only)
                if not only:
                    nc.tensor.matmul(ps[:, 0:1], sb(w_hh), prev,
                                     start=False, stop=True)
                nc.scalar.activation(dstT[:, col:col + 1], ps[:, 0:1],
                                     TANH, bias=sb(bias, 0, 1))

            def tick_b(dstT, col, w_hh, prev, bias_cols):
                # hidden matvec only; input-proj+bias comes via bias column
                ps = pspool.tile([H, 1], F32, tag="ps", name="ps")
                if prev is None:
                    nc.vector.memset(ps[:, 0:1], 0.0)
                else:
                    nc.tensor.matmul(ps[:, 0:1], sb(w_hh), prev,
                                     start=True, stop=True)
                nc.scalar.activation(dstT[:, col:col + 1], ps[:, 0:1],
                                     TANH, bias=bias_cols[:, col:col + 1])

            def make_bias(w_in, src_ap, bias, n, nm):
                # SBUF [H, n] of (W_in^T @ src + b_total) columns
                ps = pspool.tile([H, n], F32, tag="psb", name="psb",
                                 bufs=1)
                nc.tensor.matmul(ps[:, :], sb(w_in), src_ap,
                                 start=True, stop=True)
                bt = biaspool.tile([H, n], F32, tag=nm, name=nm)
                nc.vector.tensor_scalar_add(bt[:, :], ps[:, :],
                                            sb(bias, 0, 1))
                return bt

            def flush_chunk(srcT, natt, d, c, k):
                """Transpose chain cols [k*CHUNK,(k+1)*CHUNK) to natural
                layout and broadcast-DMA them to the output slice."""
                lo, hi = k * CHUNK, (k + 1) * CHUNK
                if not emit_tp:
                    return
                ps = pstpool.tile([CHUNK, H], F32, tag="pst", name="pst")
                nc.tensor.transpose(ps[:, :], srcT[:, lo:hi],
                                    sb("ident"))
                nc.vector.tensor_copy(natt[lo:hi, :], ps[:, :])
                nat = natt[lo:hi, :]
                # insert stride-0 replication dim (count 96) after partitions
                src = bass.AP(nat.tensor, nat.offset,
                              [nat.ap[0], [0, 96], nat.ap[1]])
                if c == 0:  # hx half: partitions are i, replicate over j
                    dst = out_h[d, lo:hi, :, 0, :]
                else:       # hy half: partitions are j, replicate over i
                    o = out_h[d, :, lo:hi, 1, :]
                    dst = bass.AP(o.tensor, o.offset,
                                  [o.ap[1], o.ap[0], o.ap[2]])
                if emit_dma:
                    nc.gpsimd.dma_start(dst, src)

            bias_f0 = make_bias("wx_ih0", sb("xT"), "btx0", S, "bias_f0")
            bias_g0 = make_bias("wy_ih0", sb("yT"), "bty0", T, "bias_g0")

            for t in range(S):
                tick_b(fT[0], t, "wx_hh0",
                       fT[0][:, t - 1:t] if t > 0 else None, bias_f0)
                tick_b(gT[0], t, "wy_hh0",
                       gT[0][:, t - 1:t] if t > 0 else None, bias_g0)
                if t >= 1:
                    i1 = t - 1
                    tick(fT[1], i1, "wx_ih1", fT[0][:, i1:i1 + 1],
                         "wx_hh1", fT[1][:, i1 - 1:i1] if i1 > 0 else None,
                         "btx1")
                if (t + 1) % CHUNK == 0:
                    k = (t + 1) // CHUNK - 1
                    flush_chunk(fT[0], fN[0], 0, 0, k)
                    flush_chunk(gT[0], gN[0], 0, 1, k)
                if t % CHUNK == 0 and t > 0:
                    flush_chunk(fT[1], fN[1], 1, 0, t // CHUNK - 1)

            tick(fT[1], S - 1, "wx_ih1", fT[0][:, S - 1:S],
                 "wx_hh1", fT[1][:, S - 2:S - 1], "btx1")
            flush_chunk(fT[1], fN[1], 1, 0, S // CHUNK - 1)

            # g1 inputs: g0 rolled by one column; bias columns precomputable
            # in one shot once g0 is done
            g0roll = chpool.tile([H, T], F32, tag="g0r", name="g0r")
            nc.vector.tensor_copy(g0roll[:, 1:T], gT[0][:, 0:T - 1])
            nc.vector.tensor_copy(g0roll[:, 0:1], gT[0][:, T - 1:T])
            bias_g1 = make_bias("wy_ih1", g0roll[:, :], "bty1", T, "bias_g1")

            for j in range(T):
                tick_b(gT[1], j, "wy_hh1",
                       gT[1][:, j - 1:j] if j > 0 else None, bias_g1)
                if (j + 1) % CHUNK == 0:
                    flush_chunk(gT[1], gN[1], 1, 1, (j + 1) // CHUNK - 1)

    return nc


def _get_program():
    global _PROG
    if _PROG is None:
        _PROG = _build_program()
        _PROG.finalize()
    return _PROG


TRACE = False
LAST_RESULT = [None]


def kernel(x, y, Wx_ih, Wx_hh, bx_ih, bx_hh, Wy_ih, Wy_hh, by_ih, by_hh,
           batch_size=8, src_len=96, trg_len=96, **_ignored):
    x = np.asarray(x, dtype=np.float32)
    y = np.asarray(y, dtype=np.float32)
    Wx_ih = np.asarray(Wx_ih, dtype=np.float32)
    Wx_hh = np.asarray(Wx_hh, dtype=np.float32)
    Wy_ih = np.asarray(Wy_ih, dtype=np.float32)
    Wy_hh = np.asarray(Wy_hh, dtype=np.float32)
    bx_ih = np.asarray(bx_ih, dtype=np.float32)
    bx_hh = np.asarray(bx_hh, dtype=np.float32)
    by_ih = np.asarray(by_ih, dtype=np.float32)
    by_hh = np.asarray(by_hh, dtype=np.float32)

    nc = _get_program()

    parts = {"ident": np.eye(H, dtype=np.float32)}
    for d in range(D):
        parts[f"wx_ih{d}"] = Wx_ih[d]
        parts[f"wx_hh{d}"] = Wx_hh[d]
        parts[f"wy_ih{d}"] = Wy_ih[d]
        parts[f"wy_hh{d}"] = Wy_hh[d]
        parts[f"btx{d}"] = (bx_ih[d] + bx_hh[d]).reshape(H, 1)
        parts[f"bty{d}"] = (by_ih[d] + by_hh[d]).reshape(H, 1)

    in_maps = []
    for bi in range(B):
        cons = np.empty((H, NCOLS), dtype=np.float32)
        cons[:, COLS["xT"][0]:COLS["xT"][1]] = x[bi].T
        cons[:, COLS["yT"][0]:COLS["yT"][1]] = y[bi].T
        for nm, (a, b_) in COLS.items():
            if nm not in ("xT", "yT"):
                cons[:, a:b_] = parts[nm]
        in_maps.append({"consts": cons})

    res = bass_utils.run_bass_kernel_spmd(
        nc, in_maps, core_ids=list(range(B)), trace=TRACE)
    LAST_RESULT[0] = res
    return np.stack([res.results[c]["out"] for c in range(B)], axis=0)



# revision 7
# speedup vs baseline: 2.0711x; 2.0711x over previous
"""GridRNN kernel for Trainium2 (Bass/Tile), 8-core data-parallel over batch.

Structural insight: in this GridRNN, depth-0 inputs are broadcast (x over j,
y over i) and the carry-roll along j is identity on j-constant carries, so by
induction every grid cell depends on only ONE coordinate:
    out[b,d,i,j,0,:] = f_d(b,i)   (hx, independent of j)
    out[b,d,i,j,1,:] = g_d(b,j)   (hy, independent of i)
with tiny 96-step RNN chains:
    f0(i) = tanh(Wx_ih0^T x_i   + Wx_hh0^T f0(i-1) + bx0),  f0(-1)=0
    f1(i) = tanh(Wx_ih1^T f0(i) + Wx_hh1^T f1(i-1) + bx1)
    g0(j) = tanh(Wy_ih0^T y_j   + Wy_hh0^T g0(j-1) + by0)
    g1(j) = tanh(Wy_ih1^T g0((j-1)%96) + Wy_hh1^T g1(j-1) + by1)

Instead of 96 serial (matmul -> tanh) round trips per chain (latency-bound at
~700ns each), each chain is solved parallel-in-time by Jacobi fixed-point
iteration over the whole sequence:
    H <- tanh(C + W_hh^T @ shift(H)),   shift via AP offset into a 97-col tile
Contraction rate ~0.25/sweep: 13 sweeps reach ~3.5e-3 rel err (bf16 floor),
well inside the 2e-2 gate. Each sweep is 2 full-width bf16 matmuls (N=96) +
one fused tanh, so a chain costs ~13us instead of ~67us.

Output (18.9MB/core) is assembled in SBUF as [i-partition, (j, hx|hy)] tiles
so every HBM descriptor is a 24KB contiguous run (vs 512B runs if hx/hy are
written separately), hitting near line-rate on HWDGE. The hy half (same data
for every i-partition) is replicated across partitions via a tiny HBM bounce:
write g_d natural (48KB) once, read it back with a stride-0 source AP.
"""

import numpy as np
import ml_dtypes

import concourse.bass as bass
import concourse.bacc as bacc
import concourse.mybir as mybir
import concourse.tile as tile
import concourse.bass_utils as bass_utils
from concourse.tile_rust import add_dep_helper

H, S, T, D, B = 128, 96, 96, 2, 8
NITER = 13       # Jacobi sweeps from zero state
USE_BOUNCE_DEP = False  # Tile tracks the scratch RAW via DMA sems already
QJ = 24          # j-quarter width for output pipelining
NQ = T // QJ
F32 = mybir.dt.float32
BF16 = mybir.dt.bfloat16
TANH = mybir.ActivationFunctionType.Tanh
BF = ml_dtypes.bfloat16

WNAMES = ["wx_hh0", "wx_ih0", "wy_hh0", "wy_ih0",
          "wx_hh1", "wx_ih1", "wy_hh1", "wy_ih1"]
_off = 0
COLS = {}
for _nm, _w in [("xT", S), ("yT", T), ("ident", H)] + [(n, H) for n in WNAMES]:
    COLS[_nm] = (_off, _off + _w)
    _off += _w
NCOLS = _off

_PROG = None


def _build_program():
    nc = bacc.Bacc("TRN2", target_bir_lowering=False, debug=False)

    cb_h = nc.dram_tensor("consts_bf", [H, NCOLS], BF16, kind="ExternalInput")
    cf_h = nc.dram_tensor("consts_f32", [H, 4], F32, kind="ExternalInput")
    out_h = nc.dram_tensor("out", [D, S, T, 2, H], F32, kind="ExternalOutput")
    scr_h = nc.dram_tensor("scratch", [D, T, H], BF16, kind="Internal")

    with tile.TileContext(nc) as tc:
        with (
            tc.tile_pool(name="const", bufs=1) as cpool,
            tc.tile_pool(name="chains", bufs=1) as chpool,
            tc.tile_pool(name="nat", bufs=1) as natpool,
            tc.tile_pool(name="grep", bufs=2) as gpool,
            tc.tile_pool(name="ot", bufs=3) as otpool,
            tc.tile_pool(name="ps", bufs=4, space="PSUM") as pspool,
            tc.tile_pool(name="pst", bufs=2, space="PSUM") as pstpool,
        ):
            consb = cpool.tile([H, NCOLS], BF16, tag="consb", name="consb")
            consf = cpool.tile([H, 4], F32, tag="consf", name="consf")
            nc.sync.dma_start(consb[:, :], cb_h[:, :])
            nc.sync.dma_start(consf[:, :], cf_h[:, :])

            def sb(nm):
                a, b_ = COLS[nm]
                return consb[:, a:b_]

            # chain state tiles: col 0 is the permanent zero boundary state
            Ht = {c: chpool.tile([H, S + 1], BF16, tag=c, name=c)
                  for c in ["f0", "g0", "f1", "g1"]}
            nat = {c: natpool.tile([S, H], BF16, tag=f"n{c}", name=f"n{c}")
                   for c in ["f0", "g0", "f1", "g1"]}

            def jacobi_pair(specs):
                # interleave two independent chains' sweeps so engines pipeline
                for spec in specs:
                    nc.vector.memset(Ht[spec[0]][:, :], 0.0)
                gens = [jacobi_gen(*s) for s in specs]
                while True:
                    done = True
                    for it in gens:
                        try:
                            next(it)
                            done = False
                        except StopIteration:
                            pass
                    if done:
                        break

            def jacobi_gen(cname, w_hh, w_ih, rhs_in, bias_i):
                Hc = Ht[cname]
                for _ in range(NITER):
                    ps = pspool.tile([H, S], F32, tag="ps", name="ps")
                    nc.tensor.matmul(ps[:, :], sb(w_hh), Hc[:, 0:S],
                                     start=True, stop=False)
                    nc.tensor.matmul(ps[:, :], sb(w_ih), rhs_in,
                                     start=False, stop=True)
                    nc.scalar.activation(Hc[:, 1:S + 1], ps[:, :], TANH,
                                         bias=consf[:, bias_i:bias_i + 1])
                    yield

            def to_natural(cname):
                pst = pstpool.tile([S, H], BF16, tag="pst", name="pst")
                nc.tensor.transpose(pst[:, :], Ht[cname][:, 1:S + 1], sb("ident"))
                nc.vector.tensor_copy(nat[cname][:, :], pst[:, :])

            def bounce(d, gname):
                # replicate g_d natural [96,128] to all 96 partitions via HBM
                wr = nc.sync.dma_start(scr_h[d, :, :], nat[gname][:, :])
                grep_t = gpool.tile([S, T * H], BF16, tag="grep", name=f"grep{d}")
                s = scr_h[d, :, :]
                src = bass.AP(s.tensor, s.offset, [[0, S], [1, T * H]])
                rb = nc.sync.dma_start(grep_t[:, :], src)
                if USE_BOUNCE_DEP:
                    add_dep_helper(wr.ins, rb.ins, sync=True,
                                   reason="hbm bounce RAW")
                return grep_t

            def quarter(d, q, fname, grep_t):
                ot = otpool.tile([S, QJ * 2 * H], F32, tag="ot", name="ot")
                fn = nat[fname][:, :]
                src_f = bass.AP(fn.tensor, fn.offset,
                                [fn.ap[0], [0, QJ], fn.ap[1]])
                dst_f = bass.AP(ot.tensor, ot.offset,
                                [ot.ap[0], [2 * H, QJ], [1, H]])
                nc.vector.tensor_copy(dst_f, src_f)
                gsl = grep_t[:, q * QJ * H:(q + 1) * QJ * H]
                src_g = bass.AP(gsl.tensor, gsl.offset,
                                [gsl.ap[0], [H, QJ], [1, H]])
                dst_g = bass.AP(ot.tensor, ot.offset + H,
                                [ot.ap[0], [2 * H, QJ], [1, H]])
                nc.vector.tensor_copy(dst_g, src_g)
                o = out_h[d, :, q * QJ:(q + 1) * QJ, :, :]
                src_o = bass.AP(ot.tensor, ot.offset,
                                [ot.ap[0], [2 * H, QJ], [H, 2], [1, H]])
                nc.sync.dma_start(o, src_o)

            # ---- depth 0 ----
            jacobi_pair([
                ("f0", "wx_hh0", "wx_ih0", sb("xT"), 0),
                ("g0", "wy_hh0", "wy_ih0", sb("yT"), 1),
            ])
            # roll fix: g1's input at j is g0[(j-1)%96]; col 0 := g0[95]
            nc.vector.tensor_copy(Ht["g0"][:, 0:1], Ht["g0"][:, S:S + 1])
            to_natural("f0")
            to_natural("g0")
            grep0 = bounce(0, "g0")

            # ---- depth 1 (PE/ACT) runs while depth-0 output streams (DVE/DMA)
            jacobi_pair([
                ("f1", "wx_hh1", "wx_ih1", Ht["f0"][:, 1:S + 1], 2),
                ("g1", "wy_hh1", "wy_ih1", Ht["g0"][:, 0:S], 3),
            ])

            for q in range(NQ):
                quarter(0, q, "f0", grep0)

            to_natural("f1")
            to_natural("g1")
            grep1 = bounce(1, "g1")
            for q in range(NQ):
                quarter(1, q, "f1", grep1)

    return nc


def _get_program():
    global _PROG
    if _PROG is None:
        _PROG = _build_program()
        _PROG.finalize()
    return _PROG


TRACE = False
LAST_RESULT = [None]


def kernel(x, y, Wx_ih, Wx_hh, bx_ih, bx_hh, Wy_ih, Wy_hh, by_ih, by_hh,
           batch_size=8, src_len=96, trg_len=96, **_ignored):
    x = np.asarray(x, dtype=np.float32)
    y = np.asarray(y, dtype=np.float32)

    nc = _get_program()

    wparts = {
        "ident": np.eye(H, dtype=BF),
        "wx_hh0": np.asarray(Wx_hh, np.float32)[0].astype(BF),
        "wx_ih0": np.asarray(Wx_ih, np.float32)[0].astype(BF),
        "wy_hh0": np.asarray(Wy_hh, np.float32)[0].astype(BF),
        "wy_ih0": np.asarray(Wy_ih, np.float32)[0].astype(BF),
        "wx_hh1": np.asarray(Wx_hh, np.float32)[1].astype(BF),
        "wx_ih1": np.asarray(Wx_ih, np.float32)[1].astype(BF),
        "wy_hh1": np.asarray(Wy_hh, np.float32)[1].astype(BF),
        "wy_ih1": np.asarray(Wy_ih, np.float32)[1].astype(BF),
    }
    biases = np.stack([
        np.asarray(bx_ih, np.float32)[0] + np.asarray(bx_hh, np.float32)[0],
        np.asarray(by_ih, np.float32)[0] + np.asarray(by_hh, np.float32)[0],
        np.asarray(bx_ih, np.float32)[1] + np.asarray(bx_hh, np.float32)[1],
        np.asarray(by_ih, np.float32)[1] + np.asarray(by_hh, np.float32)[1],
    ], axis=1)  # [H, 4]

    in_maps = []
    for bi in range(B):
        consb = np.empty((H, NCOLS), dtype=BF)
        consb[:, COLS["xT"][0]:COLS["xT"][1]] = x[bi].T.astype(BF)
        consb[:, COLS["yT"][0]:COLS["yT"][1]] = y[bi].T.astype(BF)
        for nm, arr in wparts.items():
            a, b_ = COLS[nm]
            consb[:, a:b_] = arr
        in_maps.append({"consts_bf": consb, "consts_f32": biases})

    res = bass_utils.run_bass_kernel_spmd(
        nc, in_maps, core_ids=list(range(B)), trace=TRACE)
    LAST_RESULT[0] = res
    return np.stack([res.results[c]["out"] for c in range(B)], axis=0)
